# revision 5
# baseline (speedup 1.0000x reference)
"""Trainium2 Bass kernel for the ConvIntrinsic GNN message-passing problem.

Shapes (hardcoded): B=1, N=50000, R=5, A=8, F=16, T=32, O=8.

Strategy:
  - Shard vertices across 8 NeuronCores (6250 each, padded to 6336 = 33*192).
  - Algebraic fold (host, tiny weights only): kernel (R,A,R,A), rotated
    neighbor weights and self weights collapse into one linear map
    W_big[(xy,f) -> (o,t)] of shape [640+16, 256] applied per vertex to the
    barycentric-interpolated patch signal, plus a bias and ReLU.
  - The mesh signal table (50000 x 16) is stored bf16 in SBUF transposed,
    one feature per partition-channel, consecutive-row PAIRS along the free
    axis -> full table is gatherable by GPSIMD ap_gather with int16 pair
    indices in a single call; parity selection is folded into the
    barycentric weights (6 weight terms per (vertex, xy) instead of 3).
  - GPSIMD groups (16 partitions each) own 5 of the 40 (r,a) cells for all
    vertices, so the post-gather weighted-sum output [128=(xyblock, feature),
    vertices] is directly the PE moving operand: no transpose anywhere.
  - Per 192-vertex tile: 1 ap_gather (2880 idx/group), DVE multiply by the
    (parity-folded, 16-replicated) weights + 6-term reduce, then 2x6
    PSUM-accumulated matmuls [128x128]x[128x192] (5 patch tiles + 1
    center/self tile), ACT engine applies bias+ReLU on PSUM->SBUF.
  - Output written [256, Vpad] per core; host transposes/assembles.

  Performance note (measured on HW): the kernel is entirely bound by
  ap_gather request latency on the Q7 cores (~28 ns/index: one
  non-pipelined ~102-cycle RD_CMD per 4 indices, Cayman ReadOverlap=0).
  95,040 indices per core => ~2.7 ms; all other engines are <15% busy and
  fully hidden. dma_gather/scatter_add/custom-ucode alternatives were
  evaluated and are not viable (256B min element, int16 index range, no
  Xtensa toolchain in-container).
"""

import sys

sys.path.insert(0, "/opt/trn_rl_repo")

import numpy as np
import ml_dtypes

from concourse import bacc, tile
import concourse.mybir as mybir
from concourse.bass_utils import run_bass_kernel_spmd

BF16 = ml_dtypes.bfloat16

N = 50000
F = 16
RR = 5
A = 8
T = 32
O = 8
NC = 8

TV = 192                    # vertices per round (tile)
VS = N // NC                # 6250 vertices per core
ROUNDS = (VS + TV - 1) // TV  # 33
VNC = ROUNDS * TV           # 6336 padded vertices per core
XY = RR * A                 # 40
XYB = XY // 8               # 5 xy cells per gpsimd group
IT = TV * XYB * 3           # 2880 gather indices per group per round
MT = TV * XYB               # 960 interp columns per group per round
E2 = N // 2                 # 25000 row pairs


def _build_program():
    nc = bacc.Bacc("TRN2", target_bir_lowering=False, debug=False)
    f32 = mybir.dt.float32
    bf16 = mybir.dt.bfloat16
    i16 = mybir.dt.int16

    d_table = nc.dram_tensor("table", [128, 2 * E2], bf16, kind="ExternalInput")
    d_idx = nc.dram_tensor("idx", [128, ROUNDS * (IT // 16)], i16, kind="ExternalInput")
    d_wexp = nc.dram_tensor("wexp", [128, ROUNDS * IT * 2], bf16, kind="ExternalInput")
    d_msh = nc.dram_tensor("meshtp", [128, VNC], bf16, kind="ExternalInput")
    d_wst = nc.dram_tensor("wstat", [128, 12 * 128], bf16, kind="ExternalInput")
    d_bias = nc.dram_tensor("biasv", [128, 2], f32, kind="ExternalInput")
    d_out = nc.dram_tensor("out", [256, VNC], f32, kind="ExternalOutput")

    with nc.allow_low_precision("bf16 gather/interp pipeline"):
        with tile.TileContext(nc) as tc:
            with (
                tc.tile_pool(name="const", bufs=1) as cp,
                tc.tile_pool(name="work", bufs=2) as wp,
                tc.tile_pool(name="psum", bufs=2, space="PSUM") as pp,
            ):
                sb_tbl = cp.tile([128, 2 * E2], bf16)
                sb_idx = cp.tile([128, ROUNDS * (IT // 16)], i16)
                sb_msh = cp.tile([128, VNC], bf16)
                sb_wst = cp.tile([128, 12 * 128], bf16)
                sb_bias = cp.tile([128, 2], f32)

                nc.sync.dma_start(sb_tbl[:], d_table[:])
                nc.sync.dma_start(sb_idx[:], d_idx[:])
                nc.sync.dma_start(sb_msh[:], d_msh[:])
                nc.sync.dma_start(sb_wst[:], d_wst[:])
                nc.sync.dma_start(sb_bias[:], d_bias[:])

                for r in range(ROUNDS):
                    wexp_t = wp.tile([128, MT, 6], bf16, name="wexp_t", tag="wexp")
                    gath_t = wp.tile([128, MT, 6], bf16, name="gath_t", tag="gath")
                    interp_t = wp.tile([128, MT], bf16, name="interp_t", tag="interp")
                    out_t = wp.tile([128, 2 * TV], f32, name="out_t", tag="outt")

                    nc.sync.dma_start(
                        wexp_t[:], d_wexp[:, r * IT * 2 : (r + 1) * IT * 2]
                    )
                    nc.gpsimd.ap_gather(
                        gath_t[:],
                        sb_tbl[:],
                        sb_idx[:, r * (IT // 16) : (r + 1) * (IT // 16)],
                        channels=128,
                        num_elems=E2,
                        d=2,
                        num_idxs=IT,
                    )
                    nc.vector.tensor_mul(gath_t[:], gath_t[:], wexp_t[:])
                    nc.vector.tensor_reduce(
                        interp_t[:],
                        gath_t[:],
                        axis=mybir.AxisListType.X,
                        op=mybir.AluOpType.add,
                    )
                    for h in range(2):
                        ps = pp.tile([128, TV], f32, name="ps", tag="ps")
                        for kt in range(6):
                            if kt < 5:
                                mov = interp_t[:, kt * TV : (kt + 1) * TV]
                            else:
                                mov = sb_msh[:, r * TV : (r + 1) * TV]
                            nc.tensor.matmul(
                                ps[:],
                                sb_wst[:, (kt * 2 + h) * 128 : (kt * 2 + h + 1) * 128],
                                mov,
                                start=(kt == 0),
                                stop=(kt == 5),
                            )
                        nc.scalar.activation(
                            out_t[:, h * TV : (h + 1) * TV],
                            ps[:],
                            mybir.ActivationFunctionType.Relu,
                            bias=sb_bias[:, h : h + 1],
                        )
                    for h in range(2):
                        nc.sync.dma_start(
                            d_out[h * 128 : (h + 1) * 128, r * TV : (r + 1) * TV],
                            out_t[:, h * TV : (h + 1) * TV],
                        )

    nc.compile()
    return nc


def _host_prep(mesh_signal, bary_coordinates, neighbor_weights, self_weights, bias, kernel):
    """Builds the per-core input maps. Only weight folding (tiny tensors) and
    layout/sharding transforms of the big inputs happen here."""
    mesh = np.asarray(mesh_signal)[0]          # [N, F] f32
    bary = np.asarray(bary_coordinates)[0]     # [N, R, A, 3, 2]
    nw = np.asarray(neighbor_weights)          # [T, R, A, F]
    sw = np.asarray(self_weights)              # [T, 1, F]
    bs = np.asarray(bias)                      # [T]
    ker = np.asarray(kernel)                   # [R, A, R, A]

    # ---- weight fold: W_big[(x*8+y)*16+f, o*32+t] ----
    wrot = np.stack([np.roll(nw, -o, axis=2) for o in range(O)])  # [O,T,R,A,F]
    w_big = np.einsum("raxy,otraf->xyfot", ker, wrot).reshape(XY * F, O * T)

    # stationary tiles [128, 12*128]: p = 16*g + f
    wst = np.zeros((128, 12 * 128), dtype=np.float32)
    p = np.arange(128)
    g = p // 16
    f = p % 16
    for kt in range(5):
        xy = g * XYB + kt
        rows = w_big[xy * F + f]               # [128, 256]
        for h in range(2):
            wst[:, (kt * 2 + h) * 128 : (kt * 2 + h + 1) * 128] = rows[
                :, h * 128 : (h + 1) * 128
            ]
    # center tile: self_weights[t, 0, f] at partitions p<16, broadcast over o
    ot = np.arange(O * T)
    cen = np.zeros((128, O * T), dtype=np.float32)
    cen[:F, :] = sw[ot % T, 0, :].T            # [F, 256]
    for h in range(2):
        wst[:, (5 * 2 + h) * 128 : (5 * 2 + h + 1) * 128] = cen[:, h * 128 : (h + 1) * 128]
    wst = wst.astype(BF16)

    biasv = np.zeros((128, 2), dtype=np.float32)
    for h in range(2):
        biasv[:, h] = bs[(h * 128 + np.arange(128)) % T]

    # ---- table: [128, 2*E2] bf16, one feature per channel, row pairs ----
    tbl16 = np.ascontiguousarray(mesh.T.astype(BF16))    # [16, N]
    table = np.tile(tbl16, (8, 1))                       # [128, N]

    idx_all = bary[..., 0].astype(np.int32).reshape(N, XY, 3)
    w_all = bary[..., 1].astype(np.float32).reshape(N, XY, 3)

    in_maps = []
    for s in range(NC):
        vs, ve = s * VS, (s + 1) * VS
        idx = np.zeros((VNC, XY, 3), dtype=np.int32)
        w = np.zeros((VNC, XY, 3), dtype=np.float32)
        idx[:VS] = idx_all[vs:ve]
        w[:VS] = w_all[vs:ve]

        par = (idx & 1).astype(np.float32)
        pairi = (idx >> 1).astype(np.int16)

        # order per (group, round): (l, n_local, j)
        pr = pairi.reshape(ROUNDS, TV, 8, XYB, 3)
        pro = pr.transpose(2, 0, 3, 1, 4).reshape(8, ROUNDS, IT)
        idx_in = (
            pro.reshape(8, ROUNDS, IT // 16, 16)
            .transpose(0, 3, 1, 2)
            .reshape(128, ROUNDS * (IT // 16))
        )
        idx_in = np.ascontiguousarray(idx_in)

        # parity-folded weights, 2 terms per (m, j): k==par gets w, else 0
        k2 = np.arange(2, dtype=np.float32)
        we6 = w[..., None] * (par[..., None] == k2)      # [VNC, XY, 3, 2]
        wr = we6.reshape(ROUNDS, TV, 8, XYB, 3, 2)
        wro = wr.transpose(2, 0, 3, 1, 4, 5).reshape(8, ROUNDS * IT * 2)
        wexp_in = np.ascontiguousarray(
            np.repeat(wro.astype(BF16), 16, axis=0)
        )  # [128, ROUNDS*IT*2]

        msh = np.zeros((128, VNC), dtype=BF16)
        msh[:F, :VS] = tbl16[:, vs:ve]

        in_maps.append(
            dict(
                table=table,
                idx=idx_in,
                wexp=wexp_in,
                meshtp=msh,
                wstat=wst,
                biasv=biasv,
            )
        )
    return in_maps


_PROGRAM_CACHE = {}


def _get_program():
    if "nc" not in _PROGRAM_CACHE:
        _PROGRAM_CACHE["nc"] = _build_program()
    return _PROGRAM_CACHE["nc"]


def kernel(mesh_signal, bary_coordinates, neighbor_weights, self_weights, bias, kernel,
           _trace=False, _core_ids=None):
    nc = _get_program()
    in_maps = _host_prep(
        mesh_signal, bary_coordinates, neighbor_weights, self_weights, bias, kernel
    )
    core_ids = list(range(NC)) if _core_ids is None else _core_ids
    res = run_bass_kernel_spmd(nc, in_maps[: len(core_ids)], core_ids, trace=_trace)
    out = np.zeros((1, N, O, T), dtype=np.float32)
    for i in range(len(core_ids)):
        o = res.results[i]["out"]              # [256, VNC]
        out[0, i * VS : (i + 1) * VS] = o[:, :VS].T.reshape(VS, O, T)
    if _trace:
        globals()["kernel"]._last_exec_ns = res.exec_time_ns
    return out


# revision 8
# speedup vs baseline: 1.0103x; 1.0103x over previous
"""Trainium2 Bass kernel for the ConvIntrinsic GNN message-passing problem.

Shapes (hardcoded): B=1, N=50000, R=5, A=8, F=16, T=32, O=8.

Strategy:
  - Shard vertices across 8 NeuronCores (6250 each, padded to 6336 = 33*192).
  - Algebraic fold (host, tiny weights only): kernel (R,A,R,A), rotated
    neighbor weights and self weights collapse into one linear map
    W_big[(xy,f) -> (o,t)] of shape [640+16, 256] applied per vertex to the
    barycentric-interpolated patch signal, plus a bias and ReLU.
  - The mesh signal table (50000 x 16) is stored bf16 in SBUF transposed,
    one feature per partition-channel, consecutive-row PAIRS along the free
    axis -> full table is gatherable by GPSIMD ap_gather with int16 pair
    indices in a single call; parity selection is folded into the
    barycentric weights (6 weight terms per (vertex, xy) instead of 3).
  - GPSIMD groups (16 partitions each) own 5 of the 40 (r,a) cells for all
    vertices, so the post-gather weighted-sum output [128=(xyblock, feature),
    vertices] is directly the PE moving operand: no transpose anywhere.
  - Per 192-vertex tile: 1 ap_gather (2880 idx/group), DVE multiply by the
    (parity-folded, 16-replicated) weights + 6-term reduce, then 2x6
    PSUM-accumulated matmuls [128x128]x[128x192] (5 patch tiles + 1
    center/self tile), ACT engine applies bias+ReLU on PSUM->SBUF.
  - Output written [256, Vpad] per core; host transposes/assembles.

  Performance note (measured on HW): the kernel is entirely bound by
  ap_gather request latency on the Q7 cores (~28 ns/index: one
  non-pipelined ~102-cycle RD_CMD per 4 indices, Cayman ReadOverlap=0).
  95,040 indices per core => ~2.7 ms; all other engines are <15% busy and
  fully hidden. dma_gather/scatter_add/custom-ucode alternatives were
  evaluated and are not viable (256B min element, int16 index range, no
  Xtensa toolchain in-container).
"""

import sys

sys.path.insert(0, "/opt/trn_rl_repo")

import numpy as np
import ml_dtypes

from concourse import bacc, tile
import concourse.mybir as mybir
from concourse.bass_utils import run_bass_kernel_spmd

BF16 = ml_dtypes.bfloat16

N = 50000
F = 16
RR = 5
A = 8
T = 32
O = 8
NC = 8

TV = 192                    # vertices per round (tile)
VS = N // NC                # 6250 vertices per core
ROUNDS = (VS + TV - 1) // TV  # 33
VNC = ROUNDS * TV           # 6336 padded vertices per core
XY = RR * A                 # 40
XYB = XY // 8               # 5 xy cells per gpsimd group
IT = TV * XYB * 3           # 2880 gather indices per group per round
MT = TV * XYB               # 960 interp columns per group per round
E2 = N // 2                 # 25000 row pairs


def _build_program():
    nc = bacc.Bacc("TRN2", target_bir_lowering=False, debug=False)
    f32 = mybir.dt.float32
    bf16 = mybir.dt.bfloat16
    i16 = mybir.dt.int16

    d_table = nc.dram_tensor("table", [128, 2 * E2], bf16, kind="ExternalInput")
    d_idx = nc.dram_tensor("idx", [128, ROUNDS * (IT // 16)], i16, kind="ExternalInput")
    d_wexp = nc.dram_tensor("wexp", [128, ROUNDS * IT * 2], bf16, kind="ExternalInput")
    d_msh = nc.dram_tensor("meshtp", [128, VNC], bf16, kind="ExternalInput")
    d_wst = nc.dram_tensor("wstat", [128, 12 * 128], bf16, kind="ExternalInput")
    d_bias = nc.dram_tensor("biasv", [128, 2], f32, kind="ExternalInput")
    d_out = nc.dram_tensor("out", [256, VNC], f32, kind="ExternalOutput")

    with nc.allow_low_precision("bf16 gather/interp pipeline"):
        with tile.TileContext(nc) as tc:
            with (
                tc.tile_pool(name="const", bufs=1) as cp,
                tc.tile_pool(name="work", bufs=2) as wp,
                tc.tile_pool(name="psum", bufs=2, space="PSUM") as pp,
            ):
                sb_tbl = cp.tile([128, 2 * E2], bf16)
                sb_idx = cp.tile([128, ROUNDS * (IT // 16)], i16)
                sb_wst = cp.tile([128, 12 * 128], bf16)
                sb_bias = cp.tile([128, 2], f32)

                # idx first (small; it gates the auto-inserted gpsimd library
                # load), then the table (the only dependency of the first
                # gather). mesh rows are streamed per round instead of
                # preloaded so they don't delay the table transfer.
                nc.sync.dma_start(sb_idx[:], d_idx[:])
                nc.sync.dma_start(sb_tbl[:], d_table[:])
                nc.sync.dma_start(sb_wst[:], d_wst[:])
                nc.sync.dma_start(sb_bias[:], d_bias[:])

                for r in range(ROUNDS):
                    wexp_t = wp.tile([128, MT, 6], bf16, name="wexp_t", tag="wexp")
                    gath_t = wp.tile([128, MT, 6], bf16, name="gath_t", tag="gath")
                    interp_t = wp.tile([128, MT], bf16, name="interp_t", tag="interp")
                    msh_t = wp.tile([128, TV], bf16, name="msh_t", tag="msh")
                    out_t = wp.tile([128, 2 * TV], f32, name="out_t", tag="outt")

                    nc.sync.dma_start(
                        wexp_t[:], d_wexp[:, r * IT * 2 : (r + 1) * IT * 2]
                    )
                    nc.sync.dma_start(msh_t[:], d_msh[:, r * TV : (r + 1) * TV])
                    nc.gpsimd.ap_gather(
                        gath_t[:],
                        sb_tbl[:],
                        sb_idx[:, r * (IT // 16) : (r + 1) * (IT // 16)],
                        channels=128,
                        num_elems=E2,
                        d=2,
                        num_idxs=IT,
                    )
                    nc.vector.tensor_mul(gath_t[:], gath_t[:], wexp_t[:])
                    nc.vector.tensor_reduce(
                        interp_t[:],
                        gath_t[:],
                        axis=mybir.AxisListType.X,
                        op=mybir.AluOpType.add,
                    )
                    for h in range(2):
                        ps = pp.tile([128, TV], f32, name="ps", tag="ps")
                        for kt in range(6):
                            if kt < 5:
                                mov = interp_t[:, kt * TV : (kt + 1) * TV]
                            else:
                                mov = msh_t[:]
                            nc.tensor.matmul(
                                ps[:],
                                sb_wst[:, (kt * 2 + h) * 128 : (kt * 2 + h + 1) * 128],
                                mov,
                                start=(kt == 0),
                                stop=(kt == 5),
                            )
                        nc.scalar.activation(
                            out_t[:, h * TV : (h + 1) * TV],
                            ps[:],
                            mybir.ActivationFunctionType.Relu,
                            bias=sb_bias[:, h : h + 1],
                        )
                    for h in range(2):
                        nc.sync.dma_start(
                            d_out[h * 128 : (h + 1) * 128, r * TV : (r + 1) * TV],
                            out_t[:, h * TV : (h + 1) * TV],
                        )

    nc.compile()
    return nc


def _host_prep(mesh_signal, bary_coordinates, neighbor_weights, self_weights, bias, kernel):
    """Builds the per-core input maps. Only weight folding (tiny tensors) and
    layout/sharding transforms of the big inputs happen here."""
    mesh = np.asarray(mesh_signal)[0]          # [N, F] f32
    bary = np.asarray(bary_coordinates)[0]     # [N, R, A, 3, 2]
    nw = np.asarray(neighbor_weights)          # [T, R, A, F]
    sw = np.asarray(self_weights)              # [T, 1, F]
    bs = np.asarray(bias)                      # [T]
    ker = np.asarray(kernel)                   # [R, A, R, A]

    # ---- weight fold: W_big[(x*8+y)*16+f, o*32+t] ----
    wrot = np.stack([np.roll(nw, -o, axis=2) for o in range(O)])  # [O,T,R,A,F]
    w_big = np.einsum("raxy,otraf->xyfot", ker, wrot).reshape(XY * F, O * T)

    # stationary tiles [128, 12*128]: p = 16*g + f
    wst = np.zeros((128, 12 * 128), dtype=np.float32)
    p = np.arange(128)
    g = p // 16
    f = p % 16
    for kt in range(5):
        xy = g * XYB + kt
        rows = w_big[xy * F + f]               # [128, 256]
        for h in range(2):
            wst[:, (kt * 2 + h) * 128 : (kt * 2 + h + 1) * 128] = rows[
                :, h * 128 : (h + 1) * 128
            ]
    # center tile: self_weights[t, 0, f] at partitions p<16, broadcast over o
    ot = np.arange(O * T)
    cen = np.zeros((128, O * T), dtype=np.float32)
    cen[:F, :] = sw[ot % T, 0, :].T            # [F, 256]
    for h in range(2):
        wst[:, (5 * 2 + h) * 128 : (5 * 2 + h + 1) * 128] = cen[:, h * 128 : (h + 1) * 128]
    wst = wst.astype(BF16)

    biasv = np.zeros((128, 2), dtype=np.float32)
    for h in range(2):
        biasv[:, h] = bs[(h * 128 + np.arange(128)) % T]

    # ---- table: [128, 2*E2] bf16, one feature per channel, row pairs ----
    tbl16 = np.ascontiguousarray(mesh.T.astype(BF16))    # [16, N]
    table = np.tile(tbl16, (8, 1))                       # [128, N]

    idx_all = bary[..., 0].astype(np.int32).reshape(N, XY, 3)
    w_all = bary[..., 1].astype(np.float32).reshape(N, XY, 3)

    in_maps = []
    for s in range(NC):
        vs, ve = s * VS, (s + 1) * VS
        idx = np.zeros((VNC, XY, 3), dtype=np.int32)
        w = np.zeros((VNC, XY, 3), dtype=np.float32)
        idx[:VS] = idx_all[vs:ve]
        w[:VS] = w_all[vs:ve]

        par = (idx & 1).astype(np.float32)
        pairi = (idx >> 1).astype(np.int16)

        # order per (group, round): (l, n_local, j)
        pr = pairi.reshape(ROUNDS, TV, 8, XYB, 3)
        pro = pr.transpose(2, 0, 3, 1, 4).reshape(8, ROUNDS, IT)
        idx_in = (
            pro.reshape(8, ROUNDS, IT // 16, 16)
            .transpose(0, 3, 1, 2)
            .reshape(128, ROUNDS * (IT // 16))
        )
        idx_in = np.ascontiguousarray(idx_in)

        # parity-folded weights, 2 terms per (m, j): k==par gets w, else 0
        k2 = np.arange(2, dtype=np.float32)
        we6 = w[..., None] * (par[..., None] == k2)      # [VNC, XY, 3, 2]
        wr = we6.reshape(ROUNDS, TV, 8, XYB, 3, 2)
        wro = wr.transpose(2, 0, 3, 1, 4, 5).reshape(8, ROUNDS * IT * 2)
        wexp_in = np.ascontiguousarray(
            np.repeat(wro.astype(BF16), 16, axis=0)
        )  # [128, ROUNDS*IT*2]

        msh = np.zeros((128, VNC), dtype=BF16)
        msh[:F, :VS] = tbl16[:, vs:ve]

        in_maps.append(
            dict(
                table=table,
                idx=idx_in,
                wexp=wexp_in,
                meshtp=msh,
                wstat=wst,
                biasv=biasv,
            )
        )
    return in_maps


_PROGRAM_CACHE = {}


def _get_program():
    if "nc" not in _PROGRAM_CACHE:
        _PROGRAM_CACHE["nc"] = _build_program()
    return _PROGRAM_CACHE["nc"]


def kernel(mesh_signal, bary_coordinates, neighbor_weights, self_weights, bias, kernel,
           _trace=False, _core_ids=None):
    nc = _get_program()
    in_maps = _host_prep(
        mesh_signal, bary_coordinates, neighbor_weights, self_weights, bias, kernel
    )
    core_ids = list(range(NC)) if _core_ids is None else _core_ids
    res = run_bass_kernel_spmd(nc, in_maps[: len(core_ids)], core_ids, trace=_trace)
    out = np.zeros((1, N, O, T), dtype=np.float32)
    for i in range(len(core_ids)):
        o = res.results[i]["out"]              # [256, VNC]
        out[0, i * VS : (i + 1) * VS] = o[:, :VS].T.reshape(VS, O, T)
    if _trace:
        globals()["kernel"]._last_exec_ns = res.exec_time_ns
    return out


# revision 9
# speedup vs baseline: 1.0126x; 1.0023x over previous
"""Trainium2 Bass kernel for the ConvIntrinsic GNN message-passing problem.

Shapes (hardcoded): B=1, N=50000, R=5, A=8, F=16, T=32, O=8.

Strategy:
  - Shard vertices across 8 NeuronCores (6250 each, padded to 6336 = 33*192).
  - Algebraic fold (host, tiny weights only): kernel (R,A,R,A), rotated
    neighbor weights and self weights collapse into one linear map
    W_big[(xy,f) -> (o,t)] of shape [640+16, 256] applied per vertex to the
    barycentric-interpolated patch signal, plus a bias and ReLU.
  - The mesh signal table (50000 x 16) is stored bf16 in SBUF transposed,
    one feature per partition-channel, consecutive-row PAIRS along the free
    axis -> full table is gatherable by GPSIMD ap_gather with int16 pair
    indices in a single call; parity selection is folded into the
    barycentric weights (6 weight terms per (vertex, xy) instead of 3).
  - GPSIMD groups (16 partitions each) own 5 of the 40 (r,a) cells for all
    vertices, so the post-gather weighted-sum output [128=(xyblock, feature),
    vertices] is directly the PE moving operand: no transpose anywhere.
  - Per 192-vertex tile: 1 ap_gather (2880 idx/group), DVE multiply by the
    (parity-folded, 16-replicated) weights + 6-term reduce, then 2x6
    PSUM-accumulated matmuls [128x128]x[128x192] (5 patch tiles + 1
    center/self tile), ACT engine applies bias+ReLU on PSUM->SBUF.
  - Output written [256, Vpad] per core; host transposes/assembles.

  Performance note (measured on HW): the kernel is entirely bound by
  ap_gather request latency on the Q7 cores (~28 ns/index: one
  non-pipelined ~102-cycle RD_CMD per 4 indices, Cayman ReadOverlap=0).
  95,040 indices per core => ~2.7 ms; all other engines are <15% busy and
  fully hidden. dma_gather/scatter_add/custom-ucode alternatives were
  evaluated and are not viable (256B min element, int16 index range, no
  Xtensa toolchain in-container).
"""

import sys

sys.path.insert(0, "/opt/trn_rl_repo")

import numpy as np
import ml_dtypes

from concourse import bacc, tile
import concourse.mybir as mybir
from concourse.bass_utils import run_bass_kernel_spmd

BF16 = ml_dtypes.bfloat16

N = 50000
F = 16
RR = 5
A = 8
T = 32
O = 8
NC = 8

TV = 288                    # vertices per round (tile)
VS = N // NC                # 6250 vertices per core
ROUNDS = (VS + TV - 1) // TV  # 33
VNC = ROUNDS * TV           # 6336 padded vertices per core
XY = RR * A                 # 40
XYB = XY // 8               # 5 xy cells per gpsimd group
IT = TV * XYB * 3           # 2880 gather indices per group per round
MT = TV * XYB               # 960 interp columns per group per round
E2 = N // 2                 # 25000 row pairs


def _build_program():
    nc = bacc.Bacc("TRN2", target_bir_lowering=False, debug=False)
    f32 = mybir.dt.float32
    bf16 = mybir.dt.bfloat16
    i16 = mybir.dt.int16

    d_table = nc.dram_tensor("table", [128, 2 * E2], bf16, kind="ExternalInput")
    d_idx = nc.dram_tensor("idx", [128, ROUNDS * (IT // 16)], i16, kind="ExternalInput")
    d_wexp = nc.dram_tensor("wexp", [128, ROUNDS * IT * 2], bf16, kind="ExternalInput")
    d_msh = nc.dram_tensor("meshtp", [128, VNC], bf16, kind="ExternalInput")
    d_wst = nc.dram_tensor("wstat", [128, 12 * 128], bf16, kind="ExternalInput")
    d_bias = nc.dram_tensor("biasv", [128, 2], f32, kind="ExternalInput")
    d_out = nc.dram_tensor("out", [256, VNC], f32, kind="ExternalOutput")

    with nc.allow_low_precision("bf16 gather/interp pipeline"):
        with tile.TileContext(nc) as tc:
            with (
                tc.tile_pool(name="const", bufs=1) as cp,
                tc.tile_pool(name="work", bufs=2) as wp,
                tc.tile_pool(name="psum", bufs=2, space="PSUM") as pp,
            ):
                sb_tbl = cp.tile([128, 2 * E2], bf16)
                sb_idx = cp.tile([128, ROUNDS * (IT // 16)], i16)
                sb_wst = cp.tile([128, 12 * 128], bf16)
                sb_bias = cp.tile([128, 2], f32)

                # idx first (small; it gates the auto-inserted gpsimd library
                # load), then the table (the only dependency of the first
                # gather). mesh rows are streamed per round instead of
                # preloaded so they don't delay the table transfer.
                nc.sync.dma_start(sb_idx[:], d_idx[:])
                nc.sync.dma_start(sb_tbl[:], d_table[:])
                nc.sync.dma_start(sb_wst[:], d_wst[:])
                nc.sync.dma_start(sb_bias[:], d_bias[:])

                for r in range(ROUNDS):
                    wexp_t = wp.tile([128, MT, 6], bf16, name="wexp_t", tag="wexp")
                    gath_t = wp.tile([128, MT, 6], bf16, name="gath_t", tag="gath")
                    interp_t = wp.tile([128, MT], bf16, name="interp_t", tag="interp")
                    msh_t = wp.tile([128, TV], bf16, name="msh_t", tag="msh")
                    out_t = wp.tile([128, 2 * TV], f32, name="out_t", tag="outt")

                    nc.sync.dma_start(
                        wexp_t[:], d_wexp[:, r * IT * 2 : (r + 1) * IT * 2]
                    )
                    nc.sync.dma_start(msh_t[:], d_msh[:, r * TV : (r + 1) * TV])
                    nc.gpsimd.ap_gather(
                        gath_t[:],
                        sb_tbl[:],
                        sb_idx[:, r * (IT // 16) : (r + 1) * (IT // 16)],
                        channels=128,
                        num_elems=E2,
                        d=2,
                        num_idxs=IT,
                    )
                    nc.vector.tensor_mul(gath_t[:], gath_t[:], wexp_t[:])
                    nc.vector.tensor_reduce(
                        interp_t[:],
                        gath_t[:],
                        axis=mybir.AxisListType.X,
                        op=mybir.AluOpType.add,
                    )
                    for h in range(2):
                        ps = pp.tile([128, TV], f32, name="ps", tag="ps")
                        for kt in range(6):
                            if kt < 5:
                                mov = interp_t[:, kt * TV : (kt + 1) * TV]
                            else:
                                mov = msh_t[:]
                            nc.tensor.matmul(
                                ps[:],
                                sb_wst[:, (kt * 2 + h) * 128 : (kt * 2 + h + 1) * 128],
                                mov,
                                start=(kt == 0),
                                stop=(kt == 5),
                            )
                        nc.scalar.activation(
                            out_t[:, h * TV : (h + 1) * TV],
                            ps[:],
                            mybir.ActivationFunctionType.Relu,
                            bias=sb_bias[:, h : h + 1],
                        )
                    for h in range(2):
                        nc.sync.dma_start(
                            d_out[h * 128 : (h + 1) * 128, r * TV : (r + 1) * TV],
                            out_t[:, h * TV : (h + 1) * TV],
                        )

    nc.compile()
    return nc


def _host_prep(mesh_signal, bary_coordinates, neighbor_weights, self_weights, bias, kernel):
    """Builds the per-core input maps. Only weight folding (tiny tensors) and
    layout/sharding transforms of the big inputs happen here."""
    mesh = np.asarray(mesh_signal)[0]          # [N, F] f32
    bary = np.asarray(bary_coordinates)[0]     # [N, R, A, 3, 2]
    nw = np.asarray(neighbor_weights)          # [T, R, A, F]
    sw = np.asarray(self_weights)              # [T, 1, F]
    bs = np.asarray(bias)                      # [T]
    ker = np.asarray(kernel)                   # [R, A, R, A]

    # ---- weight fold: W_big[(x*8+y)*16+f, o*32+t] ----
    wrot = np.stack([np.roll(nw, -o, axis=2) for o in range(O)])  # [O,T,R,A,F]
    w_big = np.einsum("raxy,otraf->xyfot", ker, wrot).reshape(XY * F, O * T)

    # stationary tiles [128, 12*128]: p = 16*g + f
    wst = np.zeros((128, 12 * 128), dtype=np.float32)
    p = np.arange(128)
    g = p // 16
    f = p % 16
    for kt in range(5):
        xy = g * XYB + kt
        rows = w_big[xy * F + f]               # [128, 256]
        for h in range(2):
            wst[:, (kt * 2 + h) * 128 : (kt * 2 + h + 1) * 128] = rows[
                :, h * 128 : (h + 1) * 128
            ]
    # center tile: self_weights[t, 0, f] at partitions p<16, broadcast over o
    ot = np.arange(O * T)
    cen = np.zeros((128, O * T), dtype=np.float32)
    cen[:F, :] = sw[ot % T, 0, :].T            # [F, 256]
    for h in range(2):
        wst[:, (5 * 2 + h) * 128 : (5 * 2 + h + 1) * 128] = cen[:, h * 128 : (h + 1) * 128]
    wst = wst.astype(BF16)

    biasv = np.zeros((128, 2), dtype=np.float32)
    for h in range(2):
        biasv[:, h] = bs[(h * 128 + np.arange(128)) % T]

    # ---- table: [128, 2*E2] bf16, one feature per channel, row pairs ----
    tbl16 = np.ascontiguousarray(mesh.T.astype(BF16))    # [16, N]
    table = np.tile(tbl16, (8, 1))                       # [128, N]

    idx_all = bary[..., 0].astype(np.int32).reshape(N, XY, 3)
    w_all = bary[..., 1].astype(np.float32).reshape(N, XY, 3)

    in_maps = []
    for s in range(NC):
        vs, ve = s * VS, (s + 1) * VS
        idx = np.zeros((VNC, XY, 3), dtype=np.int32)
        w = np.zeros((VNC, XY, 3), dtype=np.float32)
        idx[:VS] = idx_all[vs:ve]
        w[:VS] = w_all[vs:ve]

        par = (idx & 1).astype(np.float32)
        pairi = (idx >> 1).astype(np.int16)

        # order per (group, round): (l, n_local, j)
        pr = pairi.reshape(ROUNDS, TV, 8, XYB, 3)
        pro = pr.transpose(2, 0, 3, 1, 4).reshape(8, ROUNDS, IT)
        idx_in = (
            pro.reshape(8, ROUNDS, IT // 16, 16)
            .transpose(0, 3, 1, 2)
            .reshape(128, ROUNDS * (IT // 16))
        )
        idx_in = np.ascontiguousarray(idx_in)

        # parity-folded weights, 2 terms per (m, j): k==par gets w, else 0
        k2 = np.arange(2, dtype=np.float32)
        we6 = w[..., None] * (par[..., None] == k2)      # [VNC, XY, 3, 2]
        wr = we6.reshape(ROUNDS, TV, 8, XYB, 3, 2)
        wro = wr.transpose(2, 0, 3, 1, 4, 5).reshape(8, ROUNDS * IT * 2)
        wexp_in = np.ascontiguousarray(
            np.repeat(wro.astype(BF16), 16, axis=0)
        )  # [128, ROUNDS*IT*2]

        msh = np.zeros((128, VNC), dtype=BF16)
        msh[:F, :VS] = tbl16[:, vs:ve]

        in_maps.append(
            dict(
                table=table,
                idx=idx_in,
                wexp=wexp_in,
                meshtp=msh,
                wstat=wst,
                biasv=biasv,
            )
        )
    return in_maps


_PROGRAM_CACHE = {}


def _get_program():
    if "nc" not in _PROGRAM_CACHE:
        _PROGRAM_CACHE["nc"] = _build_program()
    return _PROGRAM_CACHE["nc"]


def kernel(mesh_signal, bary_coordinates, neighbor_weights, self_weights, bias, kernel,
           _trace=False, _core_ids=None):
    nc = _get_program()
    in_maps = _host_prep(
        mesh_signal, bary_coordinates, neighbor_weights, self_weights, bias, kernel
    )
    core_ids = list(range(NC)) if _core_ids is None else _core_ids
    res = run_bass_kernel_spmd(nc, in_maps[: len(core_ids)], core_ids, trace=_trace)
    out = np.zeros((1, N, O, T), dtype=np.float32)
    for i in range(len(core_ids)):
        o = res.results[i]["out"]              # [256, VNC]
        out[0, i * VS : (i + 1) * VS] = o[:, :VS].T.reshape(VS, O, T)
    if _trace:
        globals()["kernel"]._last_exec_ns = res.exec_time_ns
    return out


# revision 11
# speedup vs baseline: 1.0173x; 1.0047x over previous
"""Trainium2 Bass kernel for the ConvIntrinsic GNN message-passing problem.

Shapes (hardcoded): B=1, N=50000, R=5, A=8, F=16, T=32, O=8.

Strategy:
  - Shard vertices across 8 NeuronCores (6250 each, padded to 6336 = 33*192).
  - Algebraic fold (host, tiny weights only): kernel (R,A,R,A), rotated
    neighbor weights and self weights collapse into one linear map
    W_big[(xy,f) -> (o,t)] of shape [640+16, 256] applied per vertex to the
    barycentric-interpolated patch signal, plus a bias and ReLU.
  - The mesh signal table (50000 x 16) is stored bf16 in SBUF transposed,
    one feature per partition-channel, consecutive-row PAIRS along the free
    axis -> full table is gatherable by GPSIMD ap_gather with int16 pair
    indices in a single call; parity selection is folded into the
    barycentric weights (6 weight terms per (vertex, xy) instead of 3).
  - GPSIMD groups (16 partitions each) own 5 of the 40 (r,a) cells for all
    vertices, so the post-gather weighted-sum output [128=(xyblock, feature),
    vertices] is directly the PE moving operand: no transpose anywhere.
  - Per 192-vertex tile: 1 ap_gather (2880 idx/group), DVE multiply by the
    (parity-folded, 16-replicated) weights + 6-term reduce, then 2x6
    PSUM-accumulated matmuls [128x128]x[128x192] (5 patch tiles + 1
    center/self tile), ACT engine applies bias+ReLU on PSUM->SBUF.
  - Output written [256, Vpad] per core; host transposes/assembles.

  Performance note (measured on HW): the kernel is entirely bound by
  ap_gather request latency on the Q7 cores (~28 ns/index: one
  non-pipelined ~102-cycle RD_CMD per 4 indices, Cayman ReadOverlap=0).
  95,040 indices per core => ~2.7 ms; all other engines are <15% busy and
  fully hidden. dma_gather/scatter_add/custom-ucode alternatives were
  evaluated and are not viable (256B min element, int16 index range, no
  Xtensa toolchain in-container).
"""

import sys

sys.path.insert(0, "/opt/trn_rl_repo")

import numpy as np
import ml_dtypes

from concourse import bacc, tile
import concourse.mybir as mybir
from concourse.bass_utils import run_bass_kernel_spmd

BF16 = ml_dtypes.bfloat16

N = 50000
F = 16
RR = 5
A = 8
T = 32
O = 8
NC = 8

TV = 224                    # vertices per round (tile). Constraint: the
                            # gather ucode pops indices as 32-bit words, so
                            # IT = TV*15 must be %32 -> TV %32 == 0. 28*224
                            # = 6272 keeps gathered padding at 22 vertices.
VS = N // NC                # 6250 vertices per core
ROUNDS = (VS + TV - 1) // TV  # 33
VNC = ROUNDS * TV           # 6336 padded vertices per core
XY = RR * A                 # 40
XYB = XY // 8               # 5 xy cells per gpsimd group
IT = TV * XYB * 3           # 2880 gather indices per group per round
MT = TV * XYB               # 960 interp columns per group per round
E2 = N // 2                 # 25000 row pairs


def _build_program():
    nc = bacc.Bacc("TRN2", target_bir_lowering=False, debug=False)
    f32 = mybir.dt.float32
    bf16 = mybir.dt.bfloat16
    i16 = mybir.dt.int16

    d_table = nc.dram_tensor("table", [128, 2 * E2], bf16, kind="ExternalInput")
    d_idx = nc.dram_tensor("idx", [128, ROUNDS * (IT // 16)], i16, kind="ExternalInput")
    d_wexp = nc.dram_tensor("wexp", [128, ROUNDS * IT * 2], bf16, kind="ExternalInput")
    d_msh = nc.dram_tensor("meshtp", [128, VNC], bf16, kind="ExternalInput")
    d_wst = nc.dram_tensor("wstat", [128, 12 * 128], bf16, kind="ExternalInput")
    d_bias = nc.dram_tensor("biasv", [128, 2], f32, kind="ExternalInput")
    d_out = nc.dram_tensor("out", [256, VNC], f32, kind="ExternalOutput")

    with nc.allow_low_precision("bf16 gather/interp pipeline"):
        with tile.TileContext(nc) as tc:
            with (
                tc.tile_pool(name="const", bufs=1) as cp,
                tc.tile_pool(name="work", bufs=2) as wp,
                tc.tile_pool(name="psum", bufs=2, space="PSUM") as pp,
            ):
                sb_tbl = cp.tile([128, 2 * E2], bf16)
                sb_idx = cp.tile([128, ROUNDS * (IT // 16)], i16)
                sb_wst = cp.tile([128, 12 * 128], bf16)
                sb_bias = cp.tile([128, 2], f32)

                # idx first (small; it gates the auto-inserted gpsimd library
                # load), then the table (the only dependency of the first
                # gather). mesh rows are streamed per round instead of
                # preloaded so they don't delay the table transfer.
                nc.sync.dma_start(sb_idx[:], d_idx[:])
                nc.sync.dma_start(sb_tbl[:], d_table[:])
                nc.sync.dma_start(sb_wst[:], d_wst[:])
                nc.sync.dma_start(sb_bias[:], d_bias[:])

                for r in range(ROUNDS):
                    wexp_t = wp.tile([128, MT, 6], bf16, name="wexp_t", tag="wexp")
                    gath_t = wp.tile([128, MT, 6], bf16, name="gath_t", tag="gath")
                    interp_t = wp.tile([128, MT], bf16, name="interp_t", tag="interp")
                    msh_t = wp.tile([128, TV], bf16, name="msh_t", tag="msh")
                    out_t = wp.tile([128, 2 * TV], f32, name="out_t", tag="outt")

                    nc.sync.dma_start(
                        wexp_t[:], d_wexp[:, r * IT * 2 : (r + 1) * IT * 2]
                    )
                    nc.sync.dma_start(msh_t[:], d_msh[:, r * TV : (r + 1) * TV])
                    nc.gpsimd.ap_gather(
                        gath_t[:],
                        sb_tbl[:],
                        sb_idx[:, r * (IT // 16) : (r + 1) * (IT // 16)],
                        channels=128,
                        num_elems=E2,
                        d=2,
                        num_idxs=IT,
                    )
                    nc.vector.tensor_mul(gath_t[:], gath_t[:], wexp_t[:])
                    nc.vector.tensor_reduce(
                        interp_t[:],
                        gath_t[:],
                        axis=mybir.AxisListType.X,
                        op=mybir.AluOpType.add,
                    )
                    for h in range(2):
                        ps = pp.tile([128, TV], f32, name="ps", tag="ps")
                        for kt in range(6):
                            if kt < 5:
                                mov = interp_t[:, kt * TV : (kt + 1) * TV]
                            else:
                                mov = msh_t[:]
                            nc.tensor.matmul(
                                ps[:],
                                sb_wst[:, (kt * 2 + h) * 128 : (kt * 2 + h + 1) * 128],
                                mov,
                                start=(kt == 0),
                                stop=(kt == 5),
                            )
                        nc.scalar.activation(
                            out_t[:, h * TV : (h + 1) * TV],
                            ps[:],
                            mybir.ActivationFunctionType.Relu,
                            bias=sb_bias[:, h : h + 1],
                        )
                    for h in range(2):
                        nc.sync.dma_start(
                            d_out[h * 128 : (h + 1) * 128, r * TV : (r + 1) * TV],
                            out_t[:, h * TV : (h + 1) * TV],
                        )

    nc.compile()
    return nc


def _host_prep(mesh_signal, bary_coordinates, neighbor_weights, self_weights, bias, kernel):
    """Builds the per-core input maps. Only weight folding (tiny tensors) and
    layout/sharding transforms of the big inputs happen here."""
    mesh = np.asarray(mesh_signal)[0]          # [N, F] f32
    bary = np.asarray(bary_coordinates)[0]     # [N, R, A, 3, 2]
    nw = np.asarray(neighbor_weights)          # [T, R, A, F]
    sw = np.asarray(self_weights)              # [T, 1, F]
    bs = np.asarray(bias)                      # [T]
    ker = np.asarray(kernel)                   # [R, A, R, A]

    # ---- weight fold: W_big[(x*8+y)*16+f, o*32+t] ----
    wrot = np.stack([np.roll(nw, -o, axis=2) for o in range(O)])  # [O,T,R,A,F]
    w_big = np.einsum("raxy,otraf->xyfot", ker, wrot).reshape(XY * F, O * T)

    # stationary tiles [128, 12*128]: p = 16*g + f
    wst = np.zeros((128, 12 * 128), dtype=np.float32)
    p = np.arange(128)
    g = p // 16
    f = p % 16
    for kt in range(5):
        xy = g * XYB + kt
        rows = w_big[xy * F + f]               # [128, 256]
        for h in range(2):
            wst[:, (kt * 2 + h) * 128 : (kt * 2 + h + 1) * 128] = rows[
                :, h * 128 : (h + 1) * 128
            ]
    # center tile: self_weights[t, 0, f] at partitions p<16, broadcast over o
    ot = np.arange(O * T)
    cen = np.zeros((128, O * T), dtype=np.float32)
    cen[:F, :] = sw[ot % T, 0, :].T            # [F, 256]
    for h in range(2):
        wst[:, (5 * 2 + h) * 128 : (5 * 2 + h + 1) * 128] = cen[:, h * 128 : (h + 1) * 128]
    wst = wst.astype(BF16)

    biasv = np.zeros((128, 2), dtype=np.float32)
    for h in range(2):
        biasv[:, h] = bs[(h * 128 + np.arange(128)) % T]

    # ---- table: [128, 2*E2] bf16, one feature per channel, row pairs ----
    tbl16 = np.ascontiguousarray(mesh.T.astype(BF16))    # [16, N]
    table = np.tile(tbl16, (8, 1))                       # [128, N]

    idx_all = bary[..., 0].astype(np.int32).reshape(N, XY, 3)
    w_all = bary[..., 1].astype(np.float32).reshape(N, XY, 3)

    in_maps = []
    for s in range(NC):
        vs, ve = s * VS, (s + 1) * VS
        idx = np.zeros((VNC, XY, 3), dtype=np.int32)
        w = np.zeros((VNC, XY, 3), dtype=np.float32)
        idx[:VS] = idx_all[vs:ve]
        w[:VS] = w_all[vs:ve]

        par = (idx & 1).astype(np.float32)
        pairi = (idx >> 1).astype(np.int16)

        # order per (group, round): (l, n_local, j)
        pr = pairi.reshape(ROUNDS, TV, 8, XYB, 3)
        pro = pr.transpose(2, 0, 3, 1, 4).reshape(8, ROUNDS, IT)
        idx_in = (
            pro.reshape(8, ROUNDS, IT // 16, 16)
            .transpose(0, 3, 1, 2)
            .reshape(128, ROUNDS * (IT // 16))
        )
        idx_in = np.ascontiguousarray(idx_in)

        # parity-folded weights, 2 terms per (m, j): k==par gets w, else 0
        k2 = np.arange(2, dtype=np.float32)
        we6 = w[..., None] * (par[..., None] == k2)      # [VNC, XY, 3, 2]
        wr = we6.reshape(ROUNDS, TV, 8, XYB, 3, 2)
        wro = wr.transpose(2, 0, 3, 1, 4, 5).reshape(8, ROUNDS * IT * 2)
        wexp_in = np.ascontiguousarray(
            np.repeat(wro.astype(BF16), 16, axis=0)
        )  # [128, ROUNDS*IT*2]

        msh = np.zeros((128, VNC), dtype=BF16)
        msh[:F, :VS] = tbl16[:, vs:ve]

        in_maps.append(
            dict(
                table=table,
                idx=idx_in,
                wexp=wexp_in,
                meshtp=msh,
                wstat=wst,
                biasv=biasv,
            )
        )
    return in_maps


_PROGRAM_CACHE = {}


def _get_program():
    if "nc" not in _PROGRAM_CACHE:
        _PROGRAM_CACHE["nc"] = _build_program()
    return _PROGRAM_CACHE["nc"]


def kernel(mesh_signal, bary_coordinates, neighbor_weights, self_weights, bias, kernel,
           _trace=False, _core_ids=None):
    nc = _get_program()
    in_maps = _host_prep(
        mesh_signal, bary_coordinates, neighbor_weights, self_weights, bias, kernel
    )
    core_ids = list(range(NC)) if _core_ids is None else _core_ids
    res = run_bass_kernel_spmd(nc, in_maps[: len(core_ids)], core_ids, trace=_trace)
    out = np.zeros((1, N, O, T), dtype=np.float32)
    for i in range(len(core_ids)):
        o = res.results[i]["out"]              # [256, VNC]
        out[0, i * VS : (i + 1) * VS] = o[:, :VS].T.reshape(VS, O, T)
    if _trace:
        globals()["kernel"]._last_exec_ns = res.exec_time_ns
    return out


# revision 13
# speedup vs baseline: 1.0245x; 1.0070x over previous
"""Trainium2 Bass kernel for the ConvIntrinsic GNN message-passing problem.

Shapes (hardcoded): B=1, N=50000, R=5, A=8, F=16, T=32, O=8.

Strategy:
  - Shard vertices across 8 NeuronCores (6250 each, padded to 6336 = 33*192).
  - Algebraic fold (host, tiny weights only): kernel (R,A,R,A), rotated
    neighbor weights and self weights collapse into one linear map
    W_big[(xy,f) -> (o,t)] of shape [640+16, 256] applied per vertex to the
    barycentric-interpolated patch signal, plus a bias and ReLU.
  - The mesh signal table (50000 x 16) is stored bf16 in SBUF transposed,
    one feature per partition-channel, consecutive-row PAIRS along the free
    axis -> full table is gatherable by GPSIMD ap_gather with int16 pair
    indices in a single call; parity selection is folded into the
    barycentric weights (6 weight terms per (vertex, xy) instead of 3).
  - GPSIMD groups (16 partitions each) own 5 of the 40 (r,a) cells for all
    vertices, so the post-gather weighted-sum output [128=(xyblock, feature),
    vertices] is directly the PE moving operand: no transpose anywhere.
  - Per 192-vertex tile: 1 ap_gather (2880 idx/group), DVE multiply by the
    (parity-folded, 16-replicated) weights + 6-term reduce, then 2x6
    PSUM-accumulated matmuls [128x128]x[128x192] (5 patch tiles + 1
    center/self tile), ACT engine applies bias+ReLU on PSUM->SBUF.
  - Output written [256, Vpad] per core; host transposes/assembles.

  Performance note (measured on HW): the kernel is entirely bound by
  ap_gather request latency on the Q7 cores (~28 ns/index: one
  non-pipelined ~102-cycle RD_CMD per 4 indices, Cayman ReadOverlap=0).
  95,040 indices per core => ~2.7 ms; all other engines are <15% busy and
  fully hidden. dma_gather/scatter_add/custom-ucode alternatives were
  evaluated and are not viable (256B min element, int16 index range, no
  Xtensa toolchain in-container).
"""

import sys

sys.path.insert(0, "/opt/trn_rl_repo")

import numpy as np
import ml_dtypes

from concourse import bacc, tile
import concourse.mybir as mybir
from concourse.bass_utils import run_bass_kernel_spmd

BF16 = ml_dtypes.bfloat16

N = 50000
F = 16
RR = 5
A = 8
T = 32
O = 8
NC = 8

TV = 448                    # vertices per round (tile). Constraints: the
                            # gather ucode pops indices as 32-bit words, so
                            # IT = TV*15 must be %32 -> TV %32 == 0; matmul
                            # moving dim TV <= 512. 14*448 = 6272 keeps
                            # gathered padding at 22 vertices and halves the
                            # ~2us/call fixed cost vs TV=224. SBUF fits with
                            # wexp/interp/msh/out single-buffered (only
                            # gath_t needs bufs=2: the wexp DMA has ~170us
                            # of gather-window slack to land).
VS = N // NC                # 6250 vertices per core
ROUNDS = (VS + TV - 1) // TV  # 33
VNC = ROUNDS * TV           # 6336 padded vertices per core
XY = RR * A                 # 40
XYB = XY // 8               # 5 xy cells per gpsimd group
IT = TV * XYB * 3           # 2880 gather indices per group per round
MT = TV * XYB               # 960 interp columns per group per round
E2 = N // 2                 # 25000 row pairs


def _build_program():
    nc = bacc.Bacc("TRN2", target_bir_lowering=False, debug=False)
    f32 = mybir.dt.float32
    bf16 = mybir.dt.bfloat16
    i16 = mybir.dt.int16

    d_table = nc.dram_tensor("table", [128, 2 * E2], bf16, kind="ExternalInput")
    d_idx = nc.dram_tensor("idx", [128, ROUNDS * (IT // 16)], i16, kind="ExternalInput")
    d_wexp = nc.dram_tensor("wexp", [128, ROUNDS * IT * 2], bf16, kind="ExternalInput")
    d_msh = nc.dram_tensor("meshtp", [128, VNC], bf16, kind="ExternalInput")
    d_wst = nc.dram_tensor("wstat", [128, 12 * 128], bf16, kind="ExternalInput")
    d_bias = nc.dram_tensor("biasv", [128, 2], f32, kind="ExternalInput")
    d_out = nc.dram_tensor("out", [256, VNC], f32, kind="ExternalOutput")

    with nc.allow_low_precision("bf16 gather/interp pipeline"):
        with tile.TileContext(nc) as tc:
            with (
                tc.tile_pool(name="const", bufs=1) as cp,
                tc.tile_pool(name="work", bufs=2) as wp,
                tc.tile_pool(name="psum", bufs=2, space="PSUM") as pp,
            ):
                sb_tbl = cp.tile([128, 2 * E2], bf16)
                sb_idx = cp.tile([128, ROUNDS * (IT // 16)], i16)
                sb_wst = cp.tile([128, 12 * 128], bf16)
                sb_bias = cp.tile([128, 2], f32)

                # idx first (small; it gates the auto-inserted gpsimd library
                # load), then the table (the only dependency of the first
                # gather). mesh rows are streamed per round instead of
                # preloaded so they don't delay the table transfer.
                nc.sync.dma_start(sb_idx[:], d_idx[:])
                nc.sync.dma_start(sb_tbl[:], d_table[:])
                nc.sync.dma_start(sb_wst[:], d_wst[:])
                nc.sync.dma_start(sb_bias[:], d_bias[:])

                for r in range(ROUNDS):
                    wexp_t = wp.tile([128, MT, 6], bf16, name="wexp_t", tag="wexp",
                                     bufs=1)
                    gath_t = wp.tile([128, MT, 6], bf16, name="gath_t", tag="gath")
                    interp_t = wp.tile([128, MT], bf16, name="interp_t",
                                       tag="interp", bufs=1)
                    msh_t = wp.tile([128, TV], bf16, name="msh_t", tag="msh",
                                    bufs=1)
                    out_t = wp.tile([128, 2 * TV], f32, name="out_t", tag="outt",
                                    bufs=1)

                    nc.sync.dma_start(
                        wexp_t[:], d_wexp[:, r * IT * 2 : (r + 1) * IT * 2]
                    )
                    nc.sync.dma_start(msh_t[:], d_msh[:, r * TV : (r + 1) * TV])
                    nc.gpsimd.ap_gather(
                        gath_t[:],
                        sb_tbl[:],
                        sb_idx[:, r * (IT // 16) : (r + 1) * (IT // 16)],
                        channels=128,
                        num_elems=E2,
                        d=2,
                        num_idxs=IT,
                    )
                    nc.vector.tensor_mul(gath_t[:], gath_t[:], wexp_t[:])
                    nc.vector.tensor_reduce(
                        interp_t[:],
                        gath_t[:],
                        axis=mybir.AxisListType.X,
                        op=mybir.AluOpType.add,
                    )
                    for h in range(2):
                        ps = pp.tile([128, TV], f32, name="ps", tag="ps")
                        for kt in range(6):
                            if kt < 5:
                                mov = interp_t[:, kt * TV : (kt + 1) * TV]
                            else:
                                mov = msh_t[:]
                            nc.tensor.matmul(
                                ps[:],
                                sb_wst[:, (kt * 2 + h) * 128 : (kt * 2 + h + 1) * 128],
                                mov,
                                start=(kt == 0),
                                stop=(kt == 5),
                            )
                        nc.scalar.activation(
                            out_t[:, h * TV : (h + 1) * TV],
                            ps[:],
                            mybir.ActivationFunctionType.Relu,
                            bias=sb_bias[:, h : h + 1],
                        )
                    for h in range(2):
                        nc.sync.dma_start(
                            d_out[h * 128 : (h + 1) * 128, r * TV : (r + 1) * TV],
                            out_t[:, h * TV : (h + 1) * TV],
                        )

    nc.compile()
    return nc


def _host_prep(mesh_signal, bary_coordinates, neighbor_weights, self_weights, bias, kernel):
    """Builds the per-core input maps. Only weight folding (tiny tensors) and
    layout/sharding transforms of the big inputs happen here."""
    mesh = np.asarray(mesh_signal)[0]          # [N, F] f32
    bary = np.asarray(bary_coordinates)[0]     # [N, R, A, 3, 2]
    nw = np.asarray(neighbor_weights)          # [T, R, A, F]
    sw = np.asarray(self_weights)              # [T, 1, F]
    bs = np.asarray(bias)                      # [T]
    ker = np.asarray(kernel)                   # [R, A, R, A]

    # ---- weight fold: W_big[(x*8+y)*16+f, o*32+t] ----
    wrot = np.stack([np.roll(nw, -o, axis=2) for o in range(O)])  # [O,T,R,A,F]
    w_big = np.einsum("raxy,otraf->xyfot", ker, wrot).reshape(XY * F, O * T)

    # stationary tiles [128, 12*128]: p = 16*g + f
    wst = np.zeros((128, 12 * 128), dtype=np.float32)
    p = np.arange(128)
    g = p // 16
    f = p % 16
    for kt in range(5):
        xy = g * XYB + kt
        rows = w_big[xy * F + f]               # [128, 256]
        for h in range(2):
            wst[:, (kt * 2 + h) * 128 : (kt * 2 + h + 1) * 128] = rows[
                :, h * 128 : (h + 1) * 128
            ]
    # center tile: self_weights[t, 0, f] at partitions p<16, broadcast over o
    ot = np.arange(O * T)
    cen = np.zeros((128, O * T), dtype=np.float32)
    cen[:F, :] = sw[ot % T, 0, :].T            # [F, 256]
    for h in range(2):
        wst[:, (5 * 2 + h) * 128 : (5 * 2 + h + 1) * 128] = cen[:, h * 128 : (h + 1) * 128]
    wst = wst.astype(BF16)

    biasv = np.zeros((128, 2), dtype=np.float32)
    for h in range(2):
        biasv[:, h] = bs[(h * 128 + np.arange(128)) % T]

    # ---- table: [128, 2*E2] bf16, one feature per channel, row pairs ----
    tbl16 = np.ascontiguousarray(mesh.T.astype(BF16))    # [16, N]
    table = np.tile(tbl16, (8, 1))                       # [128, N]

    idx_all = bary[..., 0].astype(np.int32).reshape(N, XY, 3)
    w_all = bary[..., 1].astype(np.float32).reshape(N, XY, 3)

    in_maps = []
    for s in range(NC):
        vs, ve = s * VS, (s + 1) * VS
        idx = np.zeros((VNC, XY, 3), dtype=np.int32)
        w = np.zeros((VNC, XY, 3), dtype=np.float32)
        idx[:VS] = idx_all[vs:ve]
        w[:VS] = w_all[vs:ve]

        par = (idx & 1).astype(np.float32)
        pairi = (idx >> 1).astype(np.int16)

        # order per (group, round): (l, n_local, j)
        pr = pairi.reshape(ROUNDS, TV, 8, XYB, 3)
        pro = pr.transpose(2, 0, 3, 1, 4).reshape(8, ROUNDS, IT)
        idx_in = (
            pro.reshape(8, ROUNDS, IT // 16, 16)
            .transpose(0, 3, 1, 2)
            .reshape(128, ROUNDS * (IT // 16))
        )
        idx_in = np.ascontiguousarray(idx_in)

        # parity-folded weights, 2 terms per (m, j): k==par gets w, else 0
        k2 = np.arange(2, dtype=np.float32)
        we6 = w[..., None] * (par[..., None] == k2)      # [VNC, XY, 3, 2]
        wr = we6.reshape(ROUNDS, TV, 8, XYB, 3, 2)
        wro = wr.transpose(2, 0, 3, 1, 4, 5).reshape(8, ROUNDS * IT * 2)
        wexp_in = np.ascontiguousarray(
            np.repeat(wro.astype(BF16), 16, axis=0)
        )  # [128, ROUNDS*IT*2]

        msh = np.zeros((128, VNC), dtype=BF16)
        msh[:F, :VS] = tbl16[:, vs:ve]

        in_maps.append(
            dict(
                table=table,
                idx=idx_in,
                wexp=wexp_in,
                meshtp=msh,
                wstat=wst,
                biasv=biasv,
            )
        )
    return in_maps


_PROGRAM_CACHE = {}


def _get_program():
    if "nc" not in _PROGRAM_CACHE:
        _PROGRAM_CACHE["nc"] = _build_program()
    return _PROGRAM_CACHE["nc"]


def kernel(mesh_signal, bary_coordinates, neighbor_weights, self_weights, bias, kernel,
           _trace=False, _core_ids=None):
    nc = _get_program()
    in_maps = _host_prep(
        mesh_signal, bary_coordinates, neighbor_weights, self_weights, bias, kernel
    )
    core_ids = list(range(NC)) if _core_ids is None else _core_ids
    res = run_bass_kernel_spmd(nc, in_maps[: len(core_ids)], core_ids, trace=_trace)
    out = np.zeros((1, N, O, T), dtype=np.float32)
    for i in range(len(core_ids)):
        o = res.results[i]["out"]              # [256, VNC]
        out[0, i * VS : (i + 1) * VS] = o[:, :VS].T.reshape(VS, O, T)
    if _trace:
        globals()["kernel"]._last_exec_ns = res.exec_time_ns
    return out


# revision 14
# speedup vs baseline: 1.0257x; 1.0012x over previous
"""Trainium2 Bass kernel for the ConvIntrinsic GNN message-passing problem.

Shapes (hardcoded): B=1, N=50000, R=5, A=8, F=16, T=32, O=8.

Strategy:
  - Shard vertices across 8 NeuronCores (6250 each, padded to 6336 = 33*192).
  - Algebraic fold (host, tiny weights only): kernel (R,A,R,A), rotated
    neighbor weights and self weights collapse into one linear map
    W_big[(xy,f) -> (o,t)] of shape [640+16, 256] applied per vertex to the
    barycentric-interpolated patch signal, plus a bias and ReLU.
  - The mesh signal table (50000 x 16) is stored bf16 in SBUF transposed,
    one feature per partition-channel, consecutive-row PAIRS along the free
    axis -> full table is gatherable by GPSIMD ap_gather with int16 pair
    indices in a single call; parity selection is folded into the
    barycentric weights (6 weight terms per (vertex, xy) instead of 3).
  - GPSIMD groups (16 partitions each) own 5 of the 40 (r,a) cells for all
    vertices, so the post-gather weighted-sum output [128=(xyblock, feature),
    vertices] is directly the PE moving operand: no transpose anywhere.
  - Per 192-vertex tile: 1 ap_gather (2880 idx/group), DVE multiply by the
    (parity-folded, 16-replicated) weights + 6-term reduce, then 2x6
    PSUM-accumulated matmuls [128x128]x[128x192] (5 patch tiles + 1
    center/self tile), ACT engine applies bias+ReLU on PSUM->SBUF.
  - Output written [256, Vpad] per core; host transposes/assembles.

  Performance note (measured on HW): the kernel is entirely bound by
  ap_gather request latency on the Q7 cores (~28 ns/index: one
  non-pipelined ~102-cycle RD_CMD per 4 indices, Cayman ReadOverlap=0).
  95,040 indices per core => ~2.7 ms; all other engines are <15% busy and
  fully hidden. dma_gather/scatter_add/custom-ucode alternatives were
  evaluated and are not viable (256B min element, int16 index range, no
  Xtensa toolchain in-container).
"""

import sys

sys.path.insert(0, "/opt/trn_rl_repo")

import numpy as np
import ml_dtypes

from concourse import bacc, tile
import concourse.mybir as mybir
from concourse.bass_utils import run_bass_kernel_spmd

BF16 = ml_dtypes.bfloat16

N = 50000
F = 16
RR = 5
A = 8
T = 32
O = 8
NC = 8

TV = 448                    # vertices per round (tile). Constraints: the
                            # gather ucode pops indices as 32-bit words, so
                            # IT = TV*15 must be %32 -> TV %32 == 0; matmul
                            # moving dim TV <= 512. 14*448 = 6272 keeps
                            # gathered padding at 22 vertices and halves the
                            # ~2us/call fixed cost vs TV=224. SBUF fits with
                            # wexp/interp/msh/out single-buffered (only
                            # gath_t needs bufs=2: the wexp DMA has ~170us
                            # of gather-window slack to land).
VS = N // NC                # 6250 vertices per core
ROUNDS = (VS + TV - 1) // TV  # 33
VNC = ROUNDS * TV           # 6336 padded vertices per core
XY = RR * A                 # 40
XYB = XY // 8               # 5 xy cells per gpsimd group
IT = TV * XYB * 3           # 2880 gather indices per group per round
MT = TV * XYB               # 960 interp columns per group per round
E2 = N // 2                 # 25000 row pairs


def _build_program():
    nc = bacc.Bacc("TRN2", target_bir_lowering=False, debug=False)
    f32 = mybir.dt.float32
    bf16 = mybir.dt.bfloat16
    i16 = mybir.dt.int16

    d_table = nc.dram_tensor("table", [128, 2 * E2], bf16, kind="ExternalInput")
    d_idx = nc.dram_tensor("idx", [128, ROUNDS * (IT // 16)], i16, kind="ExternalInput")
    d_wexp = nc.dram_tensor("wexp", [128, ROUNDS * IT * 2], bf16, kind="ExternalInput")
    d_msh = nc.dram_tensor("meshtp", [128, VNC], bf16, kind="ExternalInput")
    d_wst = nc.dram_tensor("wstat", [128, 12 * 128], bf16, kind="ExternalInput")
    d_bias = nc.dram_tensor("biasv", [128, 2], f32, kind="ExternalInput")
    d_out = nc.dram_tensor("out", [256, VNC], f32, kind="ExternalOutput")

    with nc.allow_low_precision("bf16 gather/interp pipeline"):
        with tile.TileContext(nc) as tc:
            with (
                tc.tile_pool(name="const", bufs=1) as cp,
                tc.tile_pool(name="work", bufs=2) as wp,
                tc.tile_pool(name="psum", bufs=2, space="PSUM") as pp,
            ):
                sb_tbl = cp.tile([128, 2 * E2], bf16)
                sb_idx = cp.tile([128, ROUNDS * (IT // 16)], i16)
                sb_wst = cp.tile([128, 12 * 128], bf16)
                sb_bias = cp.tile([128, 2], f32)

                # idx first (small; it gates the auto-inserted gpsimd library
                # load), then the table (the only dependency of the first
                # gather). mesh rows are streamed per round instead of
                # preloaded so they don't delay the table transfer.
                nc.sync.dma_start(sb_idx[:], d_idx[:])
                nc.sync.dma_start(sb_tbl[:], d_table[:])
                nc.sync.dma_start(sb_wst[:], d_wst[:])
                nc.sync.dma_start(sb_bias[:], d_bias[:])

                for r in range(ROUNDS):
                    wexp_t = wp.tile([128, MT, 6], bf16, name="wexp_t", tag="wexp",
                                     bufs=1)
                    gath_t = wp.tile([128, MT, 6], bf16, name="gath_t", tag="gath")
                    interp_t = wp.tile([128, MT], bf16, name="interp_t",
                                       tag="interp", bufs=1)
                    msh_t = wp.tile([128, TV], bf16, name="msh_t", tag="msh",
                                    bufs=1)
                    out_t = wp.tile([128, 2 * TV], f32, name="out_t", tag="outt",
                                    bufs=1)

                    nc.sync.dma_start(
                        wexp_t[:], d_wexp[:, r * IT * 2 : (r + 1) * IT * 2]
                    )
                    nc.sync.dma_start(msh_t[:], d_msh[:, r * TV : (r + 1) * TV])
                    nc.gpsimd.ap_gather(
                        gath_t[:],
                        sb_tbl[:],
                        sb_idx[:, r * (IT // 16) : (r + 1) * (IT // 16)],
                        channels=128,
                        num_elems=E2,
                        d=2,
                        num_idxs=IT,
                    )
                    # Chunk DVE+PE per cell-block so the downstream drains
                    # incrementally (shrinks the final round's exposed tail).
                    ps0 = pp.tile([128, TV], f32, name="ps0", tag="ps0")
                    ps1 = pp.tile([128, TV], f32, name="ps1", tag="ps1")
                    pss = (ps0, ps1)
                    for kt in range(5):
                        gk = gath_t[:, kt * TV : (kt + 1) * TV, :]
                        ik = interp_t[:, kt * TV : (kt + 1) * TV]
                        nc.vector.tensor_mul(
                            gk, gk, wexp_t[:, kt * TV : (kt + 1) * TV, :]
                        )
                        nc.vector.tensor_reduce(
                            ik, gk, axis=mybir.AxisListType.X,
                            op=mybir.AluOpType.add,
                        )
                        for h in range(2):
                            nc.tensor.matmul(
                                pss[h][:],
                                sb_wst[:, (kt * 2 + h) * 128 : (kt * 2 + h + 1) * 128],
                                ik,
                                start=(kt == 0),
                                stop=False,
                                skip_group_check=True,
                            )
                    for h in range(2):
                        nc.tensor.matmul(
                            pss[h][:],
                            sb_wst[:, (10 + h) * 128 : (11 + h) * 128],
                            msh_t[:],
                            start=False,
                            stop=True,
                            skip_group_check=True,
                        )
                        nc.scalar.activation(
                            out_t[:, h * TV : (h + 1) * TV],
                            pss[h][:],
                            mybir.ActivationFunctionType.Relu,
                            bias=sb_bias[:, h : h + 1],
                        )
                    for h in range(2):
                        nc.sync.dma_start(
                            d_out[h * 128 : (h + 1) * 128, r * TV : (r + 1) * TV],
                            out_t[:, h * TV : (h + 1) * TV],
                        )

    nc.compile()
    return nc


def _host_prep(mesh_signal, bary_coordinates, neighbor_weights, self_weights, bias, kernel):
    """Builds the per-core input maps. Only weight folding (tiny tensors) and
    layout/sharding transforms of the big inputs happen here."""
    mesh = np.asarray(mesh_signal)[0]          # [N, F] f32
    bary = np.asarray(bary_coordinates)[0]     # [N, R, A, 3, 2]
    nw = np.asarray(neighbor_weights)          # [T, R, A, F]
    sw = np.asarray(self_weights)              # [T, 1, F]
    bs = np.asarray(bias)                      # [T]
    ker = np.asarray(kernel)                   # [R, A, R, A]

    # ---- weight fold: W_big[(x*8+y)*16+f, o*32+t] ----
    wrot = np.stack([np.roll(nw, -o, axis=2) for o in range(O)])  # [O,T,R,A,F]
    w_big = np.einsum("raxy,otraf->xyfot", ker, wrot).reshape(XY * F, O * T)

    # stationary tiles [128, 12*128]: p = 16*g + f
    wst = np.zeros((128, 12 * 128), dtype=np.float32)
    p = np.arange(128)
    g = p // 16
    f = p % 16
    for kt in range(5):
        xy = g * XYB + kt
        rows = w_big[xy * F + f]               # [128, 256]
        for h in range(2):
            wst[:, (kt * 2 + h) * 128 : (kt * 2 + h + 1) * 128] = rows[
                :, h * 128 : (h + 1) * 128
            ]
    # center tile: self_weights[t, 0, f] at partitions p<16, broadcast over o
    ot = np.arange(O * T)
    cen = np.zeros((128, O * T), dtype=np.float32)
    cen[:F, :] = sw[ot % T, 0, :].T            # [F, 256]
    for h in range(2):
        wst[:, (5 * 2 + h) * 128 : (5 * 2 + h + 1) * 128] = cen[:, h * 128 : (h + 1) * 128]
    wst = wst.astype(BF16)

    biasv = np.zeros((128, 2), dtype=np.float32)
    for h in range(2):
        biasv[:, h] = bs[(h * 128 + np.arange(128)) % T]

    # ---- table: [128, 2*E2] bf16, one feature per channel, row pairs ----
    tbl16 = np.ascontiguousarray(mesh.T.astype(BF16))    # [16, N]
    table = np.tile(tbl16, (8, 1))                       # [128, N]

    idx_all = bary[..., 0].astype(np.int32).reshape(N, XY, 3)
    w_all = bary[..., 1].astype(np.float32).reshape(N, XY, 3)

    in_maps = []
    for s in range(NC):
        vs, ve = s * VS, (s + 1) * VS
        idx = np.zeros((VNC, XY, 3), dtype=np.int32)
        w = np.zeros((VNC, XY, 3), dtype=np.float32)
        idx[:VS] = idx_all[vs:ve]
        w[:VS] = w_all[vs:ve]

        par = (idx & 1).astype(np.float32)
        pairi = (idx >> 1).astype(np.int16)

        # order per (group, round): (l, n_local, j)
        pr = pairi.reshape(ROUNDS, TV, 8, XYB, 3)
        pro = pr.transpose(2, 0, 3, 1, 4).reshape(8, ROUNDS, IT)
        idx_in = (
            pro.reshape(8, ROUNDS, IT // 16, 16)
            .transpose(0, 3, 1, 2)
            .reshape(128, ROUNDS * (IT // 16))
        )
        idx_in = np.ascontiguousarray(idx_in)

        # parity-folded weights, 2 terms per (m, j): k==par gets w, else 0
        k2 = np.arange(2, dtype=np.float32)
        we6 = w[..., None] * (par[..., None] == k2)      # [VNC, XY, 3, 2]
        wr = we6.reshape(ROUNDS, TV, 8, XYB, 3, 2)
        wro = wr.transpose(2, 0, 3, 1, 4, 5).reshape(8, ROUNDS * IT * 2)
        wexp_in = np.ascontiguousarray(
            np.repeat(wro.astype(BF16), 16, axis=0)
        )  # [128, ROUNDS*IT*2]

        msh = np.zeros((128, VNC), dtype=BF16)
        msh[:F, :VS] = tbl16[:, vs:ve]

        in_maps.append(
            dict(
                table=table,
                idx=idx_in,
                wexp=wexp_in,
                meshtp=msh,
                wstat=wst,
                biasv=biasv,
            )
        )
    return in_maps


_PROGRAM_CACHE = {}


def _get_program():
    if "nc" not in _PROGRAM_CACHE:
        _PROGRAM_CACHE["nc"] = _build_program()
    return _PROGRAM_CACHE["nc"]


def kernel(mesh_signal, bary_coordinates, neighbor_weights, self_weights, bias, kernel,
           _trace=False, _core_ids=None):
    nc = _get_program()
    in_maps = _host_prep(
        mesh_signal, bary_coordinates, neighbor_weights, self_weights, bias, kernel
    )
    core_ids = list(range(NC)) if _core_ids is None else _core_ids
    res = run_bass_kernel_spmd(nc, in_maps[: len(core_ids)], core_ids, trace=_trace)
    out = np.zeros((1, N, O, T), dtype=np.float32)
    for i in range(len(core_ids)):
        o = res.results[i]["out"]              # [256, VNC]
        out[0, i * VS : (i + 1) * VS] = o[:, :VS].T.reshape(VS, O, T)
    if _trace:
        globals()["kernel"]._last_exec_ns = res.exec_time_ns
    return out
